# revision 1
# baseline (speedup 1.0000x reference)
"""Trainium2 Bass kernel for the AdreQwen2 MoE-LoRA SwiGLU MLP.

Problem (hardcoded): B=4, S=2048, H=2048, I=5504, E=8 experts, top-2
per-batch binary gating, rank-16 LoRA adapters on gate/up/down, scale 2.0.

Distribution: token-parallel across 8 NeuronCores (1024 tokens each; each
core's tokens belong to exactly one batch, so its 2 active experts are
fixed). The host pre-selects the top-2 experts per batch and folds the
LoRA adapters into the dense weights exactly (binary gates make this pure
linear algebra): W_eff = W + 2.0 * (A_e0|A_e1 @ B_e0|B_e1)^T. The device
kernel is then a pure dense SwiGLU MLP in float32r (TF32-like tensor
engine mode, 1 cycle/row). No collectives: outputs are disjoint token
slices, concatenated on the host.

Device kernel (per core):
  phase 1: hT[i, t] = silu(Wg_eff x)[i, t] * (Wu_eff x)[i, t]
           43 I-tiles x 2 token chunks, contraction over H (16 K-tiles).
  phase 2: outT[o, t] = (Wd_eff h)[o, t]
           16 H-tiles x 2 token chunks, contraction over I (43 K-tiles).
hT is spilled to device DRAM between phases; phase 2 walks I in ki-slabs
with both token chunks resident (double-buffered slab pool whose loads
overlap the phase-1 tail), reads Wd exactly once, and accumulates partial
outputs in SBUF. Measured: ~971 us HW exec, rel err ~2.5e-4, ~90% MFU.
"""

import sys
import types

import numpy as np

# ---- problem constants (must match setup_inputs) ----
B, S, H, I, E, R = 4, 2048, 2048, 5504, 8, 16
TOP_K = 2
LORA_SCALE = 32.0 / 16.0

P = 128
KH = H // P          # 16 K-tiles over H
KI = I // P          # 43 K-tiles / M-tiles over I
MH = H // P          # 16 M-tiles over H (phase 2 output)
N_CORES = 8
T = B * S            # 8192 tokens
T_CORE = T // N_CORES  # 1024 tokens per core
TCH = 512            # token chunk (matmul moving dim)
NCHUNK = T_CORE // TCH  # 2

_CACHE: dict = {}


def install_ntff_hook():
    """The antenv stub in this image lacks axon_hooks; reconstruct it so
    run_bass_kernel_spmd(trace=True) can capture NTFF profiles."""
    if "antenv.axon_hooks" in sys.modules:
        return
    try:
        mod = types.ModuleType("antenv.axon_hooks")
        mod._hook = None
        mod.set_axon_ntff_profile_hook = lambda h: setattr(mod, "_hook", h)
        mod.get_axon_ntff_profile_hook = lambda: mod._hook
        sys.modules["antenv.axon_hooks"] = mod
        from trn_agent_boot.trn_boot import _ntff_profile_via_ctypes

        mod.set_axon_ntff_profile_hook(
            _ntff_profile_via_ctypes("/opt/axon/libaxon_pjrt.so")
        )
    except Exception:
        sys.modules.pop("antenv.axon_hooks", None)


def _build_nc():
    import concourse.bacc as bacc
    import concourse.mybir as mybir
    import concourse.tile as tile
    from concourse.bass import ts

    f32 = mybir.dt.float32
    f32r = mybir.dt.float32r
    silu_fn = mybir.ActivationFunctionType.Silu

    nc = bacc.Bacc()

    x_t = nc.declare_dram_parameter("x_t", [P, KH, T_CORE], f32r, isOutput=False)
    wg_t = nc.declare_dram_parameter("wg_t", [KI, P, KH, P], f32r, isOutput=False)
    wu_t = nc.declare_dram_parameter("wu_t", [KI, P, KH, P], f32r, isOutput=False)
    wd_t = nc.declare_dram_parameter("wd_t", [MH, P, KI, P], f32r, isOutput=False)
    outT = nc.declare_dram_parameter("outT", [H, T_CORE], f32, isOutput=True)

    hT = nc.dram_tensor("hT", [P, KI, T_CORE], f32r)

    with (
        tile.TileContext(nc) as tc,
        tc.tile_pool(name="work", bufs=3) as work,
        tc.tile_pool(name="h2p", bufs=32) as h2p,
        tc.tile_pool(name="psg", bufs=2, space="PSUM") as psg,
        tc.tile_pool(name="psup", bufs=2, space="PSUM") as psup,
        tc.tile_pool(name="pso", bufs=4, space="PSUM") as pso,
    ):
        # ---------------- phase 1: hT = silu(gate) * up ----------------
        with (
            tc.tile_pool(name="p1c", bufs=1) as p1c,
            tc.tile_pool(name="wgp", bufs=3) as wgp,
            tc.tile_pool(name="wup", bufs=3) as wup,
        ):
            def load_w(pool, tag, src, mi):
                w_sb = pool.tile([P, KH, P], f32r, tag=tag, name=f"{tag}_{mi}")
                nc.sync.dma_start(w_sb[:, :8, :], src[mi][:, :8, :])
                nc.sync.dma_start(w_sb[:, 8:, :], src[mi][:, 8:, :])
                return w_sb

            # first weight tiles before the x bulk so the first chain starts early
            wg0 = wgp.tile([P, KH, P], f32r, tag="wg", name="wg_0")
            wu0 = wup.tile([P, KH, P], f32r, tag="wu", name="wu_0")
            for q in range(4):
                nc.sync.dma_start(wg0[:, 4 * q : 4 * q + 4, :], wg_t[0][:, 4 * q : 4 * q + 4, :])
                nc.sync.dma_start(wu0[:, 4 * q : 4 * q + 4, :], wu_t[0][:, 4 * q : 4 * q + 4, :])

            # chunk-0 x tiles first so the first matmul chain starts early
            x_tiles = [[None] * KH for _ in range(NCHUNK)]
            for n in range(NCHUNK):
                for ko in range(KH):
                    xt_ = p1c.tile([P, TCH], f32r, tag=f"x{n}_{ko}", name=f"x_sb_{n}_{ko}")
                    nc.sync.dma_start(xt_[:], x_t[:, ko, ts(n, TCH)])
                    x_tiles[n][ko] = xt_

            for mi in range(KI):
                wg_sb = wg0 if mi == 0 else load_w(wgp, "wg", wg_t, mi)
                wu_sb = wu0 if mi == 0 else load_w(wup, "wu", wu_t, mi)
                for n in range(NCHUNK):
                    pg = psg.tile([P, TCH], f32, tag="g", name=f"pg_{mi}_{n}")
                    for ko in range(KH):
                        nc.tensor.matmul(
                            pg[:],
                            wg_sb[:, ko, :],
                            x_tiles[n][ko][:],
                            start=(ko == 0),
                            stop=(ko == KH - 1),
                        )
                    pup = psup.tile([P, TCH], f32, tag="up", name=f"pup_{mi}_{n}")
                    for ko in range(KH):
                        nc.tensor.matmul(
                            pup[:],
                            wu_sb[:, ko, :],
                            x_tiles[n][ko][:],
                            start=(ko == 0),
                            stop=(ko == KH - 1),
                        )
                    sil = work.tile([P, TCH], f32, tag="sil", name=f"sil_{mi}_{n}")
                    nc.scalar.activation(sil[:], pg[:], silu_fn)
                    ht = work.tile([P, TCH], f32r, tag="ht", name=f"ht_{mi}_{n}")
                    nc.vector.tensor_mul(out=ht[:], in0=sil[:], in1=pup[:])
                    nc.sync.dma_start(hT[:, mi, ts(n, TCH)], ht[:])

        # ---------------- phase 2: outT = Wd h ----------------
        # ki-slab structure: both token chunks stay resident per slab, Wd is
        # read exactly once, partial outputs accumulate in SBUF. The h slab
        # pool is double-buffered (2 slabs) and lives in the outer scope, so
        # slab-0 h loads overlap the phase-1 tail.
        SLABS = [(0, 8), (8, 15), (15, 22), (22, 29), (29, 36), (36, 43)]
        with (
            tc.tile_pool(name="osbp", bufs=MH * NCHUNK) as osbp,
            tc.tile_pool(name="wdp", bufs=6) as wdp,
        ):
            out_sb = [
                osbp.tile([P, TCH], f32, tag="osb", name=f"osb_{j}")
                for j in range(MH * NCHUNK)
            ]
            h_tiles = {}
            for si, (ks, ke) in enumerate(SLABS):
                for n in range(NCHUNK):
                    for ki in range(ks, ke):
                        t = h2p.tile([P, TCH], f32r, tag="h", name=f"h_{n}_{ki}")
                        nc.sync.dma_start(t[:], hT[:, ki, ts(n, TCH)])
                        h_tiles[(n, ki)] = t
                last = si == len(SLABS) - 1
                for mh in range(MH):
                    wd_sb = wdp.tile([P, 8, P], f32r, tag="wd", name=f"wd_{si}_{mh}")
                    nc.sync.dma_start(wd_sb[:, : ke - ks, :], wd_t[mh][:, ks:ke, :])
                    for n in range(NCHUNK):
                        po = pso.tile([P, TCH], f32, tag="o", name=f"po_{si}_{mh}_{n}")
                        for j in range(ke - ks):
                            nc.tensor.matmul(
                                po[:],
                                wd_sb[:, j, :],
                                h_tiles[(n, ks + j)][:],
                                start=(j == 0),
                                stop=(j == ke - ks - 1),
                            )
                        ob = out_sb[mh * NCHUNK + n]
                        if si == 0:
                            nc.vector.tensor_copy(out=ob[:], in_=po[:])
                        else:
                            nc.vector.tensor_add(out=ob[:], in0=ob[:], in1=po[:])
                        if last:
                            nc.sync.dma_start(outT[ts(mh, P), ts(n, TCH)], ob[:])

    nc.finalize()
    return nc


def _get_nc():
    if "nc" not in _CACHE:
        _CACHE["nc"] = _build_nc()
    return _CACHE["nc"]


def _tile_kxm(w, n_m, n_k):
    """(M, K) row-major -> (n_m, P, n_k, P) with [mi, p, ko, m] = w[128mi+m, 128ko+p]."""
    return np.ascontiguousarray(w.reshape(n_m, P, n_k, P).transpose(0, 3, 2, 1))


def _prep_inputs(x, gate_values, Wg, Ag, Bg, Wu, Au, Bu, Wd, Ad, Bd):
    """Host-side expert selection, LoRA folding, sharding, and layout prep."""
    f32 = np.float32
    c = np.ascontiguousarray

    xf = np.asarray(x, f32).reshape(T, H)
    gv = np.asarray(gate_values, f32)
    idx = np.argsort(-gv, axis=1)[:, :TOP_K]  # (B, 2) top-2 experts per batch

    Wg_, Wu_, Wd_ = np.asarray(Wg, f32), np.asarray(Wu, f32), np.asarray(Wd, f32)
    Ag_, Bg_ = np.asarray(Ag, f32), np.asarray(Bg, f32)
    Au_, Bu_ = np.asarray(Au, f32), np.asarray(Bu, f32)
    Ad_, Bd_ = np.asarray(Ad, f32), np.asarray(Bd, f32)

    per_batch = []
    for b in range(B):
        es = [int(idx[b, 0]), int(idx[b, 1])]
        # exact LoRA fold: binary top-2 gates => W_eff = W + s * (A_cat @ B_cat)^T
        ag = np.concatenate([Ag_[e] for e in es], axis=1)  # (H, 2R)
        bg = np.concatenate([Bg_[e] for e in es], axis=0)  # (2R, I)
        au = np.concatenate([Au_[e] for e in es], axis=1)
        bu = np.concatenate([Bu_[e] for e in es], axis=0)
        ad = np.concatenate([Ad_[e] for e in es], axis=1)  # (I, 2R)
        bd = np.concatenate([Bd_[e] for e in es], axis=0)  # (2R, H)
        wg_eff = Wg_ + LORA_SCALE * (ag @ bg).T            # (I, H)
        wu_eff = Wu_ + LORA_SCALE * (au @ bu).T            # (I, H)
        wd_eff = Wd_ + LORA_SCALE * (ad @ bd).T            # (H, I)
        per_batch.append(
            (
                _tile_kxm(wg_eff, KI, KH),
                _tile_kxm(wu_eff, KI, KH),
                _tile_kxm(wd_eff, MH, KI),
            )
        )

    in_maps = []
    for core in range(N_CORES):
        b = core * T_CORE // S  # batch this core's tokens belong to
        xc = xf[core * T_CORE : (core + 1) * T_CORE]               # (1024, H)
        x_tl = c(xc.T.reshape(KH, P, T_CORE).transpose(1, 0, 2))   # (P, KH, 1024)
        wg_tb, wu_tb, wd_tb = per_batch[b]
        in_maps.append({"x_t": x_tl, "wg_t": wg_tb, "wu_t": wu_tb, "wd_t": wd_tb})
    return in_maps


def _run(inputs, trace=False):
    from concourse.bass_utils import run_bass_kernel_spmd

    if trace:
        install_ntff_hook()
    nc = _get_nc()
    in_maps = _prep_inputs(**inputs)
    res = None
    last_err = None
    for attempt in range(3):  # transient NRT/axon execution errors are retriable
        try:
            res = run_bass_kernel_spmd(
                nc, in_maps, core_ids=list(range(N_CORES)), trace=trace
            )
            break
        except Exception as e:
            last_err = e
    if res is None:
        raise last_err
    outs = [res.results[c]["outT"] for c in range(N_CORES)]  # (H, 1024) each
    full = np.concatenate([o.T for o in outs], axis=0)       # (T, H)
    return full.reshape(B, S, H).astype(np.float32), res


def kernel(**inputs):
    out, _ = _run(inputs, trace=False)
    return out



# revision 2
# speedup vs baseline: 1.0656x; 1.0656x over previous
"""Trainium2 Bass kernel for the AdreQwen2 MoE-LoRA SwiGLU MLP.

Problem (hardcoded): B=4, S=2048, H=2048, I=5504, E=8 experts, top-2
per-batch binary gating, rank-16 LoRA adapters on gate/up/down, scale 2.0.

Distribution: token-parallel across 8 NeuronCores (1024 tokens each; each
core's tokens belong to exactly one batch, so its 2 active experts are
fixed). The host pre-selects the top-2 experts per batch and folds the
LoRA adapters into the dense weights exactly (binary gates make this pure
linear algebra): W_eff = W + 2.0 * (A_e0|A_e1 @ B_e0|B_e1)^T. The device
kernel is then a pure dense SwiGLU MLP in bf16 (same 1 cycle/row PE rate
as f32r, but half the DMA bytes and 2x-fast FWL weight loads). No
collectives: outputs are disjoint token slices, concatenated on the host.

Device kernel (per core), fully fused — the intermediate h stays in SBUF:
  phase 1: h[i, t] = silu(Wg_eff x)[i, t] * (Wu_eff x)[i, t]
           43 I-tiles x 2 token chunks, contraction over H (16 K-tiles),
           f32 PSUM accumulate; h written to SBUF as bf16 (86KB/partition).
  phase 2: outT[o, t] = (Wd_eff h)[o, t]
           16 H-tiles x 2 token chunks, one 43-long accumulation chain
           per output tile straight out of SBUF-resident h. Wd is
           streamed (prefetched during phase 1), read exactly once.
"""

import sys
import types

import numpy as np

# ---- problem constants (must match setup_inputs) ----
B, S, H, I, E, R = 4, 2048, 2048, 5504, 8, 16
TOP_K = 2
LORA_SCALE = 32.0 / 16.0

P = 128
KH = H // P          # 16 K-tiles over H
KI = I // P          # 43 K-tiles / M-tiles over I
MH = H // P          # 16 M-tiles over H (phase 2 output)
N_CORES = 8
T = B * S            # 8192 tokens
T_CORE = T // N_CORES  # 1024 tokens per core
TCH = 512            # token chunk (matmul moving dim)
NCHUNK = T_CORE // TCH  # 2

_CACHE: dict = {}


def install_ntff_hook():
    """The antenv stub in this image lacks axon_hooks; reconstruct it so
    run_bass_kernel_spmd(trace=True) can capture NTFF profiles."""
    if "antenv.axon_hooks" in sys.modules:
        return
    try:
        mod = types.ModuleType("antenv.axon_hooks")
        mod._hook = None
        mod.set_axon_ntff_profile_hook = lambda h: setattr(mod, "_hook", h)
        mod.get_axon_ntff_profile_hook = lambda: mod._hook
        sys.modules["antenv.axon_hooks"] = mod
        from trn_agent_boot.trn_boot import _ntff_profile_via_ctypes

        mod.set_axon_ntff_profile_hook(
            _ntff_profile_via_ctypes("/opt/axon/libaxon_pjrt.so")
        )
    except Exception:
        sys.modules.pop("antenv.axon_hooks", None)


def _build_nc():
    import concourse.bacc as bacc
    import concourse.mybir as mybir
    import concourse.tile as tile
    from concourse.bass import ts

    f32 = mybir.dt.float32
    bf16 = mybir.dt.bfloat16
    silu_fn = mybir.ActivationFunctionType.Silu

    nc = bacc.Bacc()

    x_t = nc.declare_dram_parameter("x_t", [P, KH, T_CORE], bf16, isOutput=False)
    wg_t = nc.declare_dram_parameter("wg_t", [KI, P, KH, P], bf16, isOutput=False)
    wu_t = nc.declare_dram_parameter("wu_t", [KI, P, KH, P], bf16, isOutput=False)
    wd_t = nc.declare_dram_parameter("wd_t", [MH, P, KI, P], bf16, isOutput=False)
    outT = nc.declare_dram_parameter("outT", [H, T_CORE], f32, isOutput=True)

    with (
        tile.TileContext(nc) as tc,
        tc.tile_pool(name="xp", bufs=1) as xp,
        tc.tile_pool(name="hp", bufs=1) as hp,
        tc.tile_pool(name="work", bufs=3) as work,
        tc.tile_pool(name="wgp", bufs=3) as wgp,
        tc.tile_pool(name="wup", bufs=3) as wup,
        tc.tile_pool(name="wdp", bufs=3) as wdp,
        tc.tile_pool(name="outp", bufs=4) as outp,
        tc.tile_pool(name="psg", bufs=2, space="PSUM") as psg,
        tc.tile_pool(name="psup", bufs=2, space="PSUM") as psup,
        tc.tile_pool(name="pso", bufs=4, space="PSUM") as pso,
    ):
        # ---------------- phase 1: h = silu(gate) * up (h stays in SBUF) ----
        # First weight tile is loaded in ko-quarters so the first K-chain can
        # begin as soon as quarter 0 + x[0][0] land (~2.5us), not after the
        # whole bulk.
        wg0 = wgp.tile([P, KH, P], bf16, tag="wg", name="wg_0")
        wu0 = wup.tile([P, KH, P], bf16, tag="wu", name="wu_0")
        for q in range(4):
            nc.sync.dma_start(wg0[:, 4 * q : 4 * q + 4, :], wg_t[0][:, 4 * q : 4 * q + 4, :])
        # chunk-0 x tiles right behind the first gate weights
        x_tiles = [[None] * KH for _ in range(NCHUNK)]
        for ko in range(KH):
            xt_ = xp.tile([P, TCH], bf16, tag=f"x0_{ko}", name=f"x_sb_0_{ko}")
            nc.sync.dma_start(xt_[:], x_t[:, ko, ts(0, TCH)])
            x_tiles[0][ko] = xt_
        for q in range(4):
            nc.sync.dma_start(wu0[:, 4 * q : 4 * q + 4, :], wu_t[0][:, 4 * q : 4 * q + 4, :])
        for ko in range(KH):
            xt_ = xp.tile([P, TCH], bf16, tag=f"x1_{ko}", name=f"x_sb_1_{ko}")
            nc.sync.dma_start(xt_[:], x_t[:, ko, ts(1, TCH)])
            x_tiles[1][ko] = xt_

        h_tiles = {}
        for mi in range(KI):
            if mi == 0:
                wg_sb, wu_sb = wg0, wu0
            else:
                wg_sb = wgp.tile([P, KH, P], bf16, tag="wg", name=f"wg_{mi}")
                nc.sync.dma_start(wg_sb[:], wg_t[mi])
                wu_sb = wup.tile([P, KH, P], bf16, tag="wu", name=f"wu_{mi}")
                nc.sync.dma_start(wu_sb[:], wu_t[mi])
            for n in range(NCHUNK):
                pg = psg.tile([P, TCH], f32, tag="g", name=f"pg_{mi}_{n}")
                for ko in range(KH):
                    nc.tensor.matmul(
                        pg[:],
                        wg_sb[:, ko, :],
                        x_tiles[n][ko][:],
                        start=(ko == 0),
                        stop=(ko == KH - 1),
                    )
                pup = psup.tile([P, TCH], f32, tag="up", name=f"pup_{mi}_{n}")
                for ko in range(KH):
                    nc.tensor.matmul(
                        pup[:],
                        wu_sb[:, ko, :],
                        x_tiles[n][ko][:],
                        start=(ko == 0),
                        stop=(ko == KH - 1),
                    )
                sil = work.tile([P, TCH], f32, tag="sil", name=f"sil_{mi}_{n}")
                nc.scalar.activation(sil[:], pg[:], silu_fn)
                ht = hp.tile([P, TCH], bf16, tag=f"h{mi}_{n}", name=f"ht_{mi}_{n}")
                nc.vector.tensor_mul(out=ht[:], in0=sil[:], in1=pup[:])
                h_tiles[(mi, n)] = ht

        # ---------------- phase 2: outT = Wd h (h read from SBUF) ----------
        # One 43-long f32-PSUM accumulation chain per (mh, chunk); Wd tiles
        # are prefetched during phase 1 (bufs=3 rolling).
        for mh in range(MH):
            wd_sb = wdp.tile([P, KI, P], bf16, tag="wd", name=f"wd_{mh}")
            nc.sync.dma_start(wd_sb[:], wd_t[mh])
            for n in range(NCHUNK):
                po = pso.tile([P, TCH], f32, tag="o", name=f"po_{mh}_{n}")
                for ki in range(KI):
                    nc.tensor.matmul(
                        po[:],
                        wd_sb[:, ki, :],
                        h_tiles[(ki, n)][:],
                        start=(ki == 0),
                        stop=(ki == KI - 1),
                    )
                ob = outp.tile([P, TCH], f32, tag="ob", name=f"ob_{mh}_{n}")
                nc.vector.tensor_copy(out=ob[:], in_=po[:])
                nc.sync.dma_start(outT[ts(mh, P), ts(n, TCH)], ob[:])

    nc.finalize()
    return nc


def _get_nc():
    if "nc" not in _CACHE:
        _CACHE["nc"] = _build_nc()
    return _CACHE["nc"]


def _tile_kxm(w, n_m, n_k):
    """(M, K) row-major -> (n_m, P, n_k, P) with [mi, p, ko, m] = w[128mi+m, 128ko+p]."""
    return np.ascontiguousarray(w.reshape(n_m, P, n_k, P).transpose(0, 3, 2, 1))


def _prep_inputs(x, gate_values, Wg, Ag, Bg, Wu, Au, Bu, Wd, Ad, Bd):
    """Host-side expert selection, LoRA folding, sharding, and layout prep."""
    import ml_dtypes

    f32 = np.float32
    bf16 = ml_dtypes.bfloat16
    c = np.ascontiguousarray

    xf = np.asarray(x, f32).reshape(T, H)
    gv = np.asarray(gate_values, f32)
    idx = np.argsort(-gv, axis=1)[:, :TOP_K]  # (B, 2) top-2 experts per batch

    Wg_, Wu_, Wd_ = np.asarray(Wg, f32), np.asarray(Wu, f32), np.asarray(Wd, f32)
    Ag_, Bg_ = np.asarray(Ag, f32), np.asarray(Bg, f32)
    Au_, Bu_ = np.asarray(Au, f32), np.asarray(Bu, f32)
    Ad_, Bd_ = np.asarray(Ad, f32), np.asarray(Bd, f32)

    per_batch = []
    for b in range(B):
        es = [int(idx[b, 0]), int(idx[b, 1])]
        # exact LoRA fold: binary top-2 gates => W_eff = W + s * (A_cat @ B_cat)^T
        ag = np.concatenate([Ag_[e] for e in es], axis=1)  # (H, 2R)
        bg = np.concatenate([Bg_[e] for e in es], axis=0)  # (2R, I)
        au = np.concatenate([Au_[e] for e in es], axis=1)
        bu = np.concatenate([Bu_[e] for e in es], axis=0)
        ad = np.concatenate([Ad_[e] for e in es], axis=1)  # (I, 2R)
        bd = np.concatenate([Bd_[e] for e in es], axis=0)  # (2R, H)
        wg_eff = (Wg_ + LORA_SCALE * (ag @ bg).T).astype(bf16)  # (I, H)
        wu_eff = (Wu_ + LORA_SCALE * (au @ bu).T).astype(bf16)  # (I, H)
        wd_eff = (Wd_ + LORA_SCALE * (ad @ bd).T).astype(bf16)  # (H, I)
        per_batch.append(
            (
                _tile_kxm(wg_eff, KI, KH),
                _tile_kxm(wu_eff, KI, KH),
                _tile_kxm(wd_eff, MH, KI),
            )
        )

    in_maps = []
    for core in range(N_CORES):
        b = core * T_CORE // S  # batch this core's tokens belong to
        xc = xf[core * T_CORE : (core + 1) * T_CORE].astype(bf16)   # (1024, H)
        x_tl = c(xc.T.reshape(KH, P, T_CORE).transpose(1, 0, 2))    # (P, KH, 1024)
        wg_tb, wu_tb, wd_tb = per_batch[b]
        in_maps.append({"x_t": x_tl, "wg_t": wg_tb, "wu_t": wu_tb, "wd_t": wd_tb})
    return in_maps


def _run(inputs, trace=False):
    from concourse.bass_utils import run_bass_kernel_spmd

    if trace:
        install_ntff_hook()
    nc = _get_nc()
    in_maps = _prep_inputs(**inputs)
    res = None
    last_err = None
    for attempt in range(3):  # transient NRT/axon execution errors are retriable
        try:
            res = run_bass_kernel_spmd(
                nc, in_maps, core_ids=list(range(N_CORES)), trace=trace
            )
            break
        except Exception as e:
            last_err = e
    if res is None:
        raise last_err
    outs = [res.results[c]["outT"] for c in range(N_CORES)]  # (H, 1024) each
    full = np.concatenate([o.T for o in outs], axis=0)       # (T, H)
    return full.reshape(B, S, H).astype(np.float32), res


def kernel(**inputs):
    out, _ = _run(inputs, trace=False)
    return out
